# revision 22
# baseline (speedup 1.0000x reference)
"""Trainium2 Bass kernel: CNN encoder (conv1d F=8, D=128 -> K=256, valid, + bias + ReLU).

Computation: out[b, l, k] = relu(b_k[k] + sum_{f,d} x[b, l+f, d] * filt[f,d] * W[f*D+d, k])
for l in [0, L-F)  (2040 windows).

Strategy (v3, measured 96.2us/iter steady-state vs 112.7us for the session-start
baseline under the same loop-difference benchmark):
  - Data-parallel: 32 batches / 8 cores = 4 batches per core. Params replicated.
  - Host folds filt into W (Wp[f,d,k] = filt[f,d]*W[f*128+d,k]) and transposes x to
    d-major (xT[b, d, l]); both cast to bf16 (validated rel_err 3.4e-3 << 2e-2 gate).
  - Weight reuse: the 32 output tiles ([128 k] x [512 l]) are processed in 4 groups
    of 8 = (batch-pair, kh) x 4 l-stripes; the filter-tap loop is OUTER so one
    stationary weight Wp[kh,f] serves 8 back-to-back matmuls (one per PSUM bank).
    Two mechanisms make this stick:
      1. explicit dependency chaining of all matmuls (the Tile scheduler otherwise
         reorders them tile-major, putting a weight switch between every matmul);
      2. post-compile BIR surgery dropping the redundant Ldweights the legalizer
         emits per-matmul (it never dedups; bf16 legally supports one Ldweights
         feeding several non-self-loading matmuls, unlike f32r where that path
         faults the exec unit).
    Measured A/B: grp bf16 96.2us, tile-major bf16 97.4us, tile-major f32r
    (fused self-loading matmuls) 122.4us, fused-bf16 + walrus ldw-opt 95.9us,
    4B-aligned moving operands (ALIGN=1, second shifted x copy) 96.9us.
    Per-MM cost sits at ~375ns = the isolated-MM latency (398+512)/2.4 and is
    invariant to LDW handling, schedule order, dtype and operand alignment --
    consecutive N=512 matmuls do not pipeline fill/drain here (matches the K18
    finding in the tensor-engine guide); the remaining ~40% over the 213ns
    streaming roofline is per-instruction hardware overhead, not
    schedule-fixable. Untried residual ideas: stripping the per-matmul
    semaphore increments (~26ns each, needs rewriting every downstream wait
    threshold), fp8 DoubleRow (fails the 2e-2 gate at 0.043).
  - DMA: one 512KB input DMA per batch; output evicted as bf16 (halves traffic;
    host upcasts) with one DMA per (batch, kh); all transfers alternate between
    the two HWDGE rings (qSPDynamicHW via nc.sync, qActDynamicHW via nc.scalar) --
    a single ring sustains only ~110GB/s and serializes behind 12MB/iter, which
    was the true bottleneck of the session-start baseline (its f32r compute
    stream was fully DMA-cloaked).
  - Eviction fuses bias-add + ReLU, alternating ScalarE/VectorE; output written
    k-major ([b, k, l]); host transposes back to [b, l, k].
"""

import json
import os

import numpy as np

import concourse.bacc as bacc
import concourse.bass as bass
import concourse.tile as tile
import concourse.mybir as mybir
from concourse.bass_utils import run_bass_kernel_spmd
from concourse.tile_rust import add_dep_helper

F32 = mybir.dt.float32
BF16 = mybir.dt.bfloat16

N_CORES = 8
B, L, D = 32, 2048, 128
F, K = 8, 256
N_WIN = L - F            # 2040
BP = B // N_CORES        # batches per core
KH = K // 128            # k halves
SUPERS = [(0, 512), (512, 512), (1024, 512), (1536, N_WIN - 1536)]

# matmul dtype: bf16 (legalized to Ldweights+Matmult pairs) or f32r (fused
# self-loading matmuls, 4-byte weight load internal to the PE)
MM_DT = os.environ.get("MMDT", "bf16")
W_DT = mybir.dt.float32r if MM_DT == "f32r" else BF16
X_DT = W_DT
O_DT = BF16 if os.environ.get("ODT", "bf16") == "bf16" else F32

WARM_N = int(os.environ.get("WARM_N", "16"))
# pin the PE instruction order with explicit dep edges (weight reuse depends
# on the f-outer order surviving the Tile scheduler)
PIN = os.environ.get("PIN", "1") == "1"
# grp: f-outer 8-bank interleave (1 LDW per 8 MMs, but cross-bank MMs don't
#      pipeline: ~379ns/MM isolated latency)
# tm:  tile-major (8 same-bank MMs chained per PSUM bank, LDW between each;
#      fill/drain overlap within the bank chain)
SCHED = os.environ.get("SCHED", "grp")
# keep a second one-column-shifted copy of x so every matmul's moving operand
# starts 4-byte aligned (odd filter taps otherwise start at a 2-byte offset
# in bf16)
ALIGN = os.environ.get("ALIGN", "0") == "1"
# post-compile BIR surgery:
#   dedup: drop Ldweights that reload the already-loaded weights
#   fuse:  drop ALL explicit Ldweights and mark every matmul self-loading
#          (the fused form --enable-ldw-opt accepts; pair with LDW_OPT=1)
#   off:   ship the BIR as compiled
SURG = os.environ.get("SURG", "dedup")

if os.environ.get("LDW_OPT") == "1":
    # let walrus optimize the fused weight loads (background weight buffer)
    from concourse import bass_utils as _bu
    if not getattr(_bu, "_ldw_opt_patched", False):
        _orig_run_command = _bu.run_command

        def _patched_run_command(argv, **kw):
            argv = ["--enable-ldw-opt=true" if a == "--enable-ldw-opt=false" else a
                    for a in argv]
            return _orig_run_command(argv, **kw)

        _bu.run_command = _patched_run_command
        _bu._ldw_opt_patched = True


def _dedup_ldweights(raw: bytes) -> bytes:
    """Drop PE Ldweights whose source AP is identical to the weights already
    loaded (per basic block, resetting on self-loading matmuls). Only
    instructions with no semaphore waits/updates are dropped."""
    d = json.loads(raw)
    n_dropped = 0
    for fn in d.get("functions", []):
        for blk in fn.get("blocks", []):
            out = []
            last_w = None
            for ins in blk.get("instructions", []):
                if ins.get("engine") == "PE":
                    op = ins.get("opcode")
                    if op == "Ldweights":
                        sync = ins.get("sync_info") or {}
                        clean = not (sync.get("on_wait") or sync.get("on_update"))
                        key = json.dumps(ins.get("ins"), sort_keys=True)
                        if clean and key == last_w:
                            n_dropped += 1
                            continue
                        last_w = key
                    elif op == "Matmult":
                        if ins.get("ldweights", True):
                            last_w = None  # self-loading matmul clobbers weights
                    elif op in ("EventSemaphore",):
                        pass
                    else:
                        last_w = None  # branches/drains: be conservative
                out.append(ins)
            blk["instructions"] = out
    if os.environ.get("DEDUP_VERBOSE"):
        print(f"_dedup_ldweights: dropped {n_dropped}")
    return json.dumps(d).encode()


def _fuse_ldweights(raw: bytes) -> bytes:
    """Drop every explicit PE Ldweights and mark all matmuls self-loading
    (ldweights=true; the stationary AP is still present in ins[1]). An
    Ldweights carrying semaphore waits becomes a pure EventSemaphore so no
    synchronization is lost."""
    d = json.loads(raw)
    n_dropped = n_kept = 0
    for fn in d.get("functions", []):
        for blk in fn.get("blocks", []):
            out = []
            for ins in blk.get("instructions", []):
                if ins.get("engine") == "PE":
                    op = ins.get("opcode")
                    if op == "Ldweights":
                        sync = ins.get("sync_info") or {}
                        if sync.get("on_wait") or sync.get("on_update"):
                            ins = {k: v for k, v in ins.items()
                                   if k in ("debug", "engine", "name", "sync_info")}
                            ins["opcode"] = "EventSemaphore"
                            ins["ins"] = []
                            ins["outs"] = []
                            n_kept += 1
                        else:
                            n_dropped += 1
                            continue
                    elif op == "Matmult":
                        ins["ldweights"] = True
                out.append(ins)
            blk["instructions"] = out
    if os.environ.get("DEDUP_VERBOSE"):
        print(f"_fuse_ldweights: dropped {n_dropped}, kept {n_kept} as waits")
    return json.dumps(d).encode()


def _build_program(reps=1, loop_n=0):
    """One SPMD program for all 8 cores. loop_n>0 wraps the body in a hardware
    For_i loop (benchmarking only: every iteration rewrites the same output)."""
    nc = bacc.Bacc(
        "TRN2",
        target_bir_lowering=False,
        debug=False,
        num_devices=N_CORES,
    )
    xT_d = nc.declare_dram_parameter("xT", [BP, D, L], X_DT, isOutput=False)
    wp_d = nc.declare_dram_parameter("wp", [KH, D, F, 128], W_DT, isOutput=False)
    bias_d = nc.declare_dram_parameter("bias", [128, KH], F32, isOutput=False)
    out_d = nc.declare_dram_parameter(
        "outT", [reps * BP, KH, 128, N_WIN], O_DT, isOutput=True)

    # alternate every transfer between the two HWDGE rings
    dq_state = [0]

    def dma(dst, src):
        eng = nc.sync if dq_state[0] == 0 else nc.scalar
        dq_state[0] ^= 1
        eng.dma_start(dst, src)

    def warmup(nc, tc, pools):
        # PE HAM warm-up on junk data while the first input DMAs land
        # (plain fp32: 4 cycles/row, no f32r rounded-producer quirk).
        # Emitted once, OUTSIDE the For_i benchmark loop.
        const_pool, xt_pool, psum_pool, out_pool = pools
        warm_x = const_pool.tile([D, 64], F32, tag="warmx")
        warm_ps = psum_pool.tile([128, 512], F32, tag="ps", name="warm_ps")
        nc.gpsimd.memset(warm_x[:], 0.0)
        for _ in range(WARM_N):
            nc.tensor.matmul(warm_ps[0:64, 0:64], lhsT=warm_x[:, 0:64],
                             rhs=warm_x[:], start=True, stop=True)

    def body(nc, tc, pools, r):
        const_pool, xt_pool, psum_pool, out_pool = pools
        bias_sb = const_pool.tile([128, KH], F32, tag="bias")
        wp_sb = []
        for kh in range(KH):
            t_wp = const_pool.tile([D, F, 128], W_DT, tag=f"wp{kh}")
            wp_sb.append(t_wp)
        xt = [xt_pool.tile([D, L], X_DT, tag="xt", name=f"xt{b}")
              for b in range(BP)]
        xt_odd = [xt_pool.tile([D, L], X_DT, tag="xtodd", name=f"xtodd{b}")
                  for b in range(BP)] if ALIGN else None

        def rhs_slice(b, c, ls):
            if ALIGN and c % 2 == 1:
                return xt_odd[b][:, c - 1:c - 1 + ls]
            return xt[b][:, c:c + ls]

        dma(xt[0][:], xT_d[0])
        dma(wp_sb[0][:], wp_d[0])
        if ALIGN:
            dma(xt_odd[0][:, 0:L - 1], xT_d[0, :, 1:L])
        dma(bias_sb[:], bias_d[:])
        dma(xt[1][:], xT_d[1])
        if ALIGN:
            dma(xt_odd[1][:, 0:L - 1], xT_d[1, :, 1:L])
        dma(wp_sb[1][:], wp_d[1])
        for b in range(2, BP):
            dma(xt[b][:], xT_d[b])
            if ALIGN:
                dma(xt_odd[b][:, 0:L - 1], xT_d[b, :, 1:L])

        prev_mm = [None]

        def mm(out, lhsT, rhs, start, stop):
            i = nc.tensor.matmul(out, lhsT=lhsT, rhs=rhs, start=start,
                                 stop=stop).ins
            if PIN:
                if prev_mm[0] is not None:
                    add_dep_helper(i, prev_mm[0], reason="pin PE f-outer order")
                prev_mm[0] = i
            return i

        evictor = 0
        for bp in range(BP // 2):
            bs = (2 * bp, 2 * bp + 1)
            for kh in range(KH):
                ob = {b: out_pool.tile([128, N_WIN], O_DT, tag="ob",
                                       name=f"ob_{b}_{kh}") for b in bs}
                ps = {}
                for b in bs:
                    for si in range(4):
                        ps[b, si] = psum_pool.tile([128, 512], F32, tag="ps",
                                                   name=f"ps_{b}_{si}")
                order = [(b, si) for si in range(4) for b in bs]
                if SCHED == "grp":
                    for f in range(F):
                        for b, si in order:
                            l0, ls = SUPERS[si]
                            mm(ps[b, si][:, :ls], wp_sb[kh][:, f, :],
                               rhs_slice(b, l0 + f, ls),
                               start=(f == 0), stop=(f == F - 1))
                else:  # tile-major: full f-chain per bank before switching
                    for b, si in order:
                        l0, ls = SUPERS[si]
                        for f in range(F):
                            mm(ps[b, si][:, :ls], wp_sb[kh][:, f, :],
                               rhs_slice(b, l0 + f, ls),
                               start=(f == 0), stop=(f == F - 1))
                for b in bs:
                    for si in range(4):
                        l0, ls = SUPERS[si]
                        if evictor == 0:
                            nc.scalar.activation(
                                ob[b][:, l0:l0 + ls], ps[b, si][:, :ls],
                                mybir.ActivationFunctionType.Relu,
                                bias=bias_sb[:, kh:kh + 1], scale=1.0,
                            )
                        else:
                            nc.vector.tensor_scalar(
                                ob[b][:, l0:l0 + ls], ps[b, si][:, :ls],
                                scalar1=bias_sb[:, kh:kh + 1], scalar2=0.0,
                                op0=mybir.AluOpType.add, op1=mybir.AluOpType.max,
                            )
                        evictor ^= 1
                    dma(out_d[r * BP + b, kh], ob[b][:])

    with tile.TileContext(nc) as tc:
        with (
            tc.tile_pool(name="const", bufs=2) as const_pool,
            tc.tile_pool(name="xt", bufs=BP) as xt_pool,
            tc.tile_pool(name="psum", bufs=8, space=bass.MemorySpace.PSUM) as psum_pool,
            tc.tile_pool(name="out", bufs=4) as out_pool,
        ):
            pools = (const_pool, xt_pool, psum_pool, out_pool)
            warmup(nc, tc, pools)
            if loop_n > 0:
                with tc.For_i(0, loop_n, 1,
                              hint_engines=(mybir.EngineType.PE,)):
                    for r in range(reps):
                        body(nc, tc, pools, r)
            else:
                for r in range(reps):
                    body(nc, tc, pools, r)
    nc.compile()
    if SURG == "dedup":
        _orig = nc.to_json_bytes
        nc.to_json_bytes = lambda: _dedup_ldweights(_orig())
    elif SURG == "fuse":
        _orig = nc.to_json_bytes
        nc.to_json_bytes = lambda: _fuse_ldweights(_orig())
    return nc


def _prep_inputs(user_batch, filt, W_k, b_k):
    user_batch = np.asarray(user_batch, dtype=np.float32)
    filt = np.asarray(filt, dtype=np.float32)
    W_k = np.asarray(W_k, dtype=np.float32)
    b_k = np.asarray(b_k, dtype=np.float32)

    wp = W_k.reshape(F, D, K) * filt[:, :, None]          # [f, d, k]
    wp_host = np.ascontiguousarray(                        # [kh, d, f, 128]
        wp.reshape(F, D, KH, 128).transpose(2, 1, 0, 3))
    bias_host = np.ascontiguousarray(b_k.reshape(KH, 128).T)  # [128, kh]
    xT = np.ascontiguousarray(user_batch.transpose(0, 2, 1))  # [b, d, l]
    if W_DT == BF16:
        import ml_dtypes
        wp_host = wp_host.astype(ml_dtypes.bfloat16)
        xT = xT.astype(ml_dtypes.bfloat16)
    return xT, wp_host, bias_host


def _run(user_batch, filt, W_k, b_k, trace=False):
    xT, wp_host, bias_host = _prep_inputs(user_batch, filt, W_k, b_k)
    nc = _build_program()
    in_maps = [
        {"xT": xT[c * BP:(c + 1) * BP], "wp": wp_host, "bias": bias_host}
        for c in range(N_CORES)
    ]
    res = run_bass_kernel_spmd(nc, in_maps, list(range(N_CORES)), trace=trace)
    outT = np.concatenate(
        [np.asarray(r["outT"], dtype=np.float32) for r in res.results], axis=0)
    out = outT.reshape(B, K, N_WIN).transpose(0, 2, 1)               # [B, N_WIN, K]
    return np.ascontiguousarray(out), res


def kernel(user_batch, filt, W_k, b_k):
    out, _ = _run(user_batch, filt, W_k, b_k, trace=False)
    return out


# revision 25
# speedup vs baseline: 1.0632x; 1.0632x over previous
"""Trainium2 Bass kernel: CNN encoder (conv1d F=8, D=128 -> K=256, valid, + bias + ReLU).

Computation: out[b, l, k] = relu(b_k[k] + sum_{f,d} x[b, l+f, d] * filt[f,d] * W[f*D+d, k])
for l in [0, L-F)  (2040 windows).

Strategy (v3, measured 96.2us/iter steady-state vs 112.7us for the session-start
baseline under the same loop-difference benchmark):
  - Data-parallel: 32 batches / 8 cores = 4 batches per core. Params replicated.
  - Host folds filt into W (Wp[f,d,k] = filt[f,d]*W[f*128+d,k]) and transposes x to
    d-major (xT[b, d, l]); both cast to bf16 (validated rel_err 3.4e-3 << 2e-2 gate).
  - Weight reuse: the 32 output tiles ([128 k] x [512 l]) are processed in 4 groups
    of 8 = (batch-pair, kh) x 4 l-stripes; the filter-tap loop is OUTER so one
    stationary weight Wp[kh,f] serves 8 back-to-back matmuls (one per PSUM bank).
    Two mechanisms make this stick:
      1. explicit dependency chaining of all matmuls (the Tile scheduler otherwise
         reorders them tile-major, putting a weight switch between every matmul);
      2. post-compile BIR surgery dropping the redundant Ldweights the legalizer
         emits per-matmul (it never dedups; bf16 legally supports one Ldweights
         feeding several non-self-loading matmuls, unlike f32r where that path
         faults the exec unit).
    Measured A/B: grp bf16 96.2us, tile-major bf16 97.4us, tile-major f32r
    (fused self-loading matmuls) 122.4us, fused-bf16 + walrus ldw-opt 95.9us,
    4B-aligned moving operands (ALIGN=1, second shifted x copy) 96.9us.
    Per-MM cost sits at ~375ns = the isolated-MM latency (398+512)/2.4 and is
    invariant to LDW handling, schedule order, dtype and operand alignment --
    consecutive N=512 matmuls do not pipeline fill/drain here (matches the K18
    finding in the tensor-engine guide); the remaining ~40% over the 213ns
    streaming roofline is per-instruction hardware overhead, not
    schedule-fixable. Untried residual ideas: stripping the per-matmul
    semaphore increments (~26ns each, needs rewriting every downstream wait
    threshold), fp8 DoubleRow (fails the 2e-2 gate at 0.043).
  - DMA: one 512KB input DMA per batch; output evicted as bf16 (halves traffic;
    host upcasts) with one DMA per (batch, kh); all transfers alternate between
    the two HWDGE rings (qSPDynamicHW via nc.sync, qActDynamicHW via nc.scalar) --
    a single ring sustains only ~110GB/s and serializes behind 12MB/iter, which
    was the true bottleneck of the session-start baseline (its f32r compute
    stream was fully DMA-cloaked).
  - Eviction fuses bias-add + ReLU, alternating ScalarE/VectorE; output written
    k-major ([b, k, l]); host transposes back to [b, l, k].
"""

import json
import os

import numpy as np

import concourse.bacc as bacc
import concourse.bass as bass
import concourse.tile as tile
import concourse.mybir as mybir
from concourse.bass_utils import run_bass_kernel_spmd
from concourse.tile_rust import add_dep_helper

F32 = mybir.dt.float32
BF16 = mybir.dt.bfloat16

N_CORES = 8
B, L, D = 32, 2048, 128
F, K = 8, 256
N_WIN = L - F            # 2040
BP = B // N_CORES        # batches per core
KH = K // 128            # k halves
SUPERS = [(0, 512), (512, 512), (1024, 512), (1536, N_WIN - 1536)]

# matmul dtype: bf16 (legalized to Ldweights+Matmult pairs) or f32r (fused
# self-loading matmuls, 4-byte weight load internal to the PE)
MM_DT = os.environ.get("MMDT", "bf16")
W_DT = mybir.dt.float32r if MM_DT == "f32r" else BF16
X_DT = W_DT
O_DT = BF16 if os.environ.get("ODT", "bf16") == "bf16" else F32

WARM_N = int(os.environ.get("WARM_N", "16"))
# pin the PE instruction order with explicit dep edges (weight reuse depends
# on the f-outer order surviving the Tile scheduler)
PIN = os.environ.get("PIN", "1") == "1"
# grp: f-outer 8-bank interleave (1 LDW per 8 MMs, but cross-bank MMs don't
#      pipeline: ~379ns/MM isolated latency)
# tm:  tile-major (8 same-bank MMs chained per PSUM bank, LDW between each;
#      fill/drain overlap within the bank chain)
SCHED = os.environ.get("SCHED", "grp")
# keep a second one-column-shifted copy of x so every matmul's moving operand
# starts 4-byte aligned (odd filter taps otherwise start at a 2-byte offset
# in bf16)
ALIGN = os.environ.get("ALIGN", "0") == "1"
# post-compile BIR surgery:
#   dedup: drop Ldweights that reload the already-loaded weights
#   fuse:  drop ALL explicit Ldweights and mark every matmul self-loading
#          (the fused form --enable-ldw-opt accepts; pair with LDW_OPT=1)
#   off:   ship the BIR as compiled
SURG = os.environ.get("SURG", "dedup")

if os.environ.get("LDW_OPT") == "1":
    # let walrus optimize the fused weight loads (background weight buffer)
    from concourse import bass_utils as _bu
    if not getattr(_bu, "_ldw_opt_patched", False):
        _orig_run_command = _bu.run_command

        def _patched_run_command(argv, **kw):
            argv = ["--enable-ldw-opt=true" if a == "--enable-ldw-opt=false" else a
                    for a in argv]
            return _orig_run_command(argv, **kw)

        _bu.run_command = _patched_run_command
        _bu._ldw_opt_patched = True


def _dedup_ldweights(raw: bytes) -> bytes:
    """Drop PE Ldweights whose source AP is identical to the weights already
    loaded (per basic block, resetting on self-loading matmuls). Only
    instructions with no semaphore waits/updates are dropped."""
    d = json.loads(raw)
    n_dropped = 0
    for fn in d.get("functions", []):
        for blk in fn.get("blocks", []):
            out = []
            last_w = None
            for ins in blk.get("instructions", []):
                if ins.get("engine") == "PE":
                    op = ins.get("opcode")
                    if op == "Ldweights":
                        sync = ins.get("sync_info") or {}
                        clean = not (sync.get("on_wait") or sync.get("on_update"))
                        key = json.dumps(ins.get("ins"), sort_keys=True)
                        if clean and key == last_w:
                            n_dropped += 1
                            continue
                        last_w = key
                    elif op == "Matmult":
                        if ins.get("ldweights", True):
                            last_w = None  # self-loading matmul clobbers weights
                    elif op in ("EventSemaphore",):
                        pass
                    else:
                        last_w = None  # branches/drains: be conservative
                out.append(ins)
            blk["instructions"] = out
    if os.environ.get("DEDUP_VERBOSE"):
        print(f"_dedup_ldweights: dropped {n_dropped}")
    return json.dumps(d).encode()


def _fuse_ldweights(raw: bytes) -> bytes:
    """Drop every explicit PE Ldweights and mark all matmuls self-loading
    (ldweights=true; the stationary AP is still present in ins[1]). An
    Ldweights carrying semaphore waits becomes a pure EventSemaphore so no
    synchronization is lost."""
    d = json.loads(raw)
    n_dropped = n_kept = 0
    for fn in d.get("functions", []):
        for blk in fn.get("blocks", []):
            out = []
            for ins in blk.get("instructions", []):
                if ins.get("engine") == "PE":
                    op = ins.get("opcode")
                    if op == "Ldweights":
                        sync = ins.get("sync_info") or {}
                        if sync.get("on_wait") or sync.get("on_update"):
                            ins = {k: v for k, v in ins.items()
                                   if k in ("debug", "engine", "name", "sync_info")}
                            ins["opcode"] = "EventSemaphore"
                            ins["ins"] = []
                            ins["outs"] = []
                            n_kept += 1
                        else:
                            n_dropped += 1
                            continue
                    elif op == "Matmult":
                        ins["ldweights"] = True
                out.append(ins)
            blk["instructions"] = out
    if os.environ.get("DEDUP_VERBOSE"):
        print(f"_fuse_ldweights: dropped {n_dropped}, kept {n_kept} as waits")
    return json.dumps(d).encode()


def _strip_pe_sem_incs(d: dict) -> dict:
    """Per-matmul sem increments serialize through the EVT_SEM register
    (~26ns each). Downstream waits only reference ~33 distinct thresholds, so
    keep increments only at those release points and renumber every wait to
    its rank among the kept increments — release timing is unchanged."""
    # semaphores whose increments all come from the PE engine
    sems = set()
    for fn in d.get("functions", []):
        for blk in fn.get("blocks", []):
            for ins in blk.get("instructions", []):
                for u in (ins.get("sync_info") or {}).get("on_update", []):
                    n = u.get("ant_name", "")
                    if n.startswith("PE_") and ins.get("engine") == "PE":
                        sems.add(n)
    n_dropped = 0
    for sem in sems:
        needed = set()
        for fn in d.get("functions", []):
            for blk in fn.get("blocks", []):
                for ins in blk.get("instructions", []):
                    for w in (ins.get("sync_info") or {}).get("on_wait", []):
                        if w.get("ant_name") == sem and w.get("wait_value", 0) > 0:
                            needed.add(w["wait_value"])
        kept = sorted(needed)
        rank = {v: i + 1 for i, v in enumerate(kept)}
        # per-block original/kept unit-inc counts (for rescaling the loop
        # back-edge bulk sub / skip-path bulk add, which carry the body
        # block's full per-iteration inc count)
        orig_per_blk, kept_per_blk = {}, {}
        count = 0
        for fn in d.get("functions", []):
            for blk in fn.get("blocks", []):
                bid = id(blk)
                for ins in blk.get("instructions", []):
                    s = ins.get("sync_info")
                    if not s:
                        continue
                    new_upd = []
                    for u in s.get("on_update", []):
                        if (u.get("ant_name") == sem
                                and u.get("update_mode") == "sem-inc"
                                and u.get("update_value") == 1):
                            count += 1
                            orig_per_blk[bid] = orig_per_blk.get(bid, 0) + 1
                            if count in needed:
                                kept_per_blk[bid] = kept_per_blk.get(bid, 0) + 1
                                new_upd.append(u)
                            else:
                                n_dropped += 1
                        else:
                            new_upd.append(u)
                    s["on_update"] = new_upd
                    for w in s.get("on_wait", []):
                        if w.get("ant_name") == sem and w.get("wait_value", 0) > 0:
                            w["wait_value"] = rank[w["wait_value"]]
        for fn in d.get("functions", []):
            for blk in fn.get("blocks", []):
                for ins in blk.get("instructions", []):
                    s = ins.get("sync_info")
                    if not s:
                        continue
                    for u in s.get("on_update", []):
                        if (u.get("ant_name") == sem
                                and u.get("update_mode") in ("sem-add-imm",
                                                             "sem-sub-imm")):
                            v = u.get("update_value")
                            srcs = [b for b, n in orig_per_blk.items() if n == v]
                            assert len(srcs) == 1, (v, orig_per_blk)
                            u["update_value"] = kept_per_blk.get(srcs[0], 0)
    if os.environ.get("DEDUP_VERBOSE"):
        print(f"_strip_pe_sem_incs: dropped {n_dropped} increments "
              f"({len(sems)} sems)")
    return d


SEMSTRIP = os.environ.get("SEMSTRIP", "0") == "1"


def _build_program(reps=1, loop_n=0):
    """One SPMD program for all 8 cores. loop_n>0 wraps the body in a hardware
    For_i loop (benchmarking only: every iteration rewrites the same output)."""
    nc = bacc.Bacc(
        "TRN2",
        target_bir_lowering=False,
        debug=False,
        num_devices=N_CORES,
    )
    xT_d = nc.declare_dram_parameter("xT", [BP, D, L], X_DT, isOutput=False)
    wp_d = nc.declare_dram_parameter("wp", [KH, D, F, 128], W_DT, isOutput=False)
    bias_d = nc.declare_dram_parameter("bias", [128, KH], F32, isOutput=False)
    out_d = nc.declare_dram_parameter(
        "outT", [reps * BP, KH, 128, N_WIN], O_DT, isOutput=True)

    # alternate every transfer between the two HWDGE rings
    dq_state = [0]

    def dma(dst, src):
        eng = nc.sync if dq_state[0] == 0 else nc.scalar
        dq_state[0] ^= 1
        eng.dma_start(dst, src)

    def warmup(nc, tc, pools):
        # PE HAM warm-up on junk data while the first input DMAs land
        # (plain fp32: 4 cycles/row, no f32r rounded-producer quirk).
        # Emitted once, OUTSIDE the For_i benchmark loop.
        const_pool, xt_pool, psum_pool, out_pool = pools
        warm_x = const_pool.tile([D, 64], F32, tag="warmx")
        warm_ps = psum_pool.tile([128, 512], F32, tag="ps", name="warm_ps")
        nc.gpsimd.memset(warm_x[:], 0.0)
        for _ in range(WARM_N):
            nc.tensor.matmul(warm_ps[0:64, 0:64], lhsT=warm_x[:, 0:64],
                             rhs=warm_x[:], start=True, stop=True)

    def body(nc, tc, pools, r):
        const_pool, xt_pool, psum_pool, out_pool = pools
        bias_sb = const_pool.tile([128, KH], F32, tag="bias")
        wp_sb = []
        for kh in range(KH):
            t_wp = const_pool.tile([D, F, 128], W_DT, tag=f"wp{kh}")
            wp_sb.append(t_wp)
        xt = [xt_pool.tile([D, L], X_DT, tag="xt", name=f"xt{b}")
              for b in range(BP)]
        xt_odd = [xt_pool.tile([D, L], X_DT, tag="xtodd", name=f"xtodd{b}")
                  for b in range(BP)] if ALIGN else None

        def rhs_slice(b, c, ls):
            if ALIGN and c % 2 == 1:
                return xt_odd[b][:, c - 1:c - 1 + ls]
            return xt[b][:, c:c + ls]

        dma(xt[0][:], xT_d[0])
        dma(wp_sb[0][:], wp_d[0])
        if ALIGN:
            dma(xt_odd[0][:, 0:L - 1], xT_d[0, :, 1:L])
        dma(bias_sb[:], bias_d[:])
        dma(xt[1][:], xT_d[1])
        if ALIGN:
            dma(xt_odd[1][:, 0:L - 1], xT_d[1, :, 1:L])
        dma(wp_sb[1][:], wp_d[1])
        for b in range(2, BP):
            dma(xt[b][:], xT_d[b])
            if ALIGN:
                dma(xt_odd[b][:, 0:L - 1], xT_d[b, :, 1:L])

        prev_mm = [None]

        def mm(out, lhsT, rhs, start, stop):
            i = nc.tensor.matmul(out, lhsT=lhsT, rhs=rhs, start=start,
                                 stop=stop).ins
            if PIN:
                if prev_mm[0] is not None:
                    add_dep_helper(i, prev_mm[0], reason="pin PE f-outer order")
                prev_mm[0] = i
            return i

        evictor = 0
        for bp in range(BP // 2):
            bs = (2 * bp, 2 * bp + 1)
            for kh in range(KH):
                ob = {b: out_pool.tile([128, N_WIN], O_DT, tag="ob",
                                       name=f"ob_{b}_{kh}") for b in bs}
                ps = {}
                for b in bs:
                    for si in range(4):
                        ps[b, si] = psum_pool.tile([128, 512], F32, tag="ps",
                                                   name=f"ps_{b}_{si}")
                order = [(b, si) for si in range(4) for b in bs]
                if SCHED == "grp":
                    for f in range(F):
                        for b, si in order:
                            l0, ls = SUPERS[si]
                            mm(ps[b, si][:, :ls], wp_sb[kh][:, f, :],
                               rhs_slice(b, l0 + f, ls),
                               start=(f == 0), stop=(f == F - 1))
                else:  # tile-major: full f-chain per bank before switching
                    for b, si in order:
                        l0, ls = SUPERS[si]
                        for f in range(F):
                            mm(ps[b, si][:, :ls], wp_sb[kh][:, f, :],
                               rhs_slice(b, l0 + f, ls),
                               start=(f == 0), stop=(f == F - 1))
                for b in bs:
                    for si in range(4):
                        l0, ls = SUPERS[si]
                        if evictor == 0:
                            nc.scalar.activation(
                                ob[b][:, l0:l0 + ls], ps[b, si][:, :ls],
                                mybir.ActivationFunctionType.Relu,
                                bias=bias_sb[:, kh:kh + 1], scale=1.0,
                            )
                        else:
                            nc.vector.tensor_scalar(
                                ob[b][:, l0:l0 + ls], ps[b, si][:, :ls],
                                scalar1=bias_sb[:, kh:kh + 1], scalar2=0.0,
                                op0=mybir.AluOpType.add, op1=mybir.AluOpType.max,
                            )
                        evictor ^= 1
                    dma(out_d[r * BP + b, kh], ob[b][:])

    with tile.TileContext(nc) as tc:
        with (
            tc.tile_pool(name="const", bufs=2) as const_pool,
            tc.tile_pool(name="xt", bufs=BP) as xt_pool,
            tc.tile_pool(name="psum", bufs=8, space=bass.MemorySpace.PSUM) as psum_pool,
            tc.tile_pool(name="out", bufs=4) as out_pool,
        ):
            pools = (const_pool, xt_pool, psum_pool, out_pool)
            warmup(nc, tc, pools)
            if loop_n > 0:
                with tc.For_i(0, loop_n, 1,
                              hint_engines=(mybir.EngineType.PE,)):
                    for r in range(reps):
                        body(nc, tc, pools, r)
            else:
                for r in range(reps):
                    body(nc, tc, pools, r)
    nc.compile()
    if SURG in ("dedup", "fuse") or SEMSTRIP:
        _orig = nc.to_json_bytes

        def _surgery():
            raw = _orig()
            if SURG == "dedup":
                raw = _dedup_ldweights(raw)
            elif SURG == "fuse":
                raw = _fuse_ldweights(raw)
            if SEMSTRIP:
                raw = json.dumps(_strip_pe_sem_incs(json.loads(raw))).encode()
            return raw

        nc.to_json_bytes = _surgery
    return nc


def _prep_inputs(user_batch, filt, W_k, b_k):
    user_batch = np.asarray(user_batch, dtype=np.float32)
    filt = np.asarray(filt, dtype=np.float32)
    W_k = np.asarray(W_k, dtype=np.float32)
    b_k = np.asarray(b_k, dtype=np.float32)

    wp = W_k.reshape(F, D, K) * filt[:, :, None]          # [f, d, k]
    wp_host = np.ascontiguousarray(                        # [kh, d, f, 128]
        wp.reshape(F, D, KH, 128).transpose(2, 1, 0, 3))
    bias_host = np.ascontiguousarray(b_k.reshape(KH, 128).T)  # [128, kh]
    xT = np.ascontiguousarray(user_batch.transpose(0, 2, 1))  # [b, d, l]
    if W_DT == BF16:
        import ml_dtypes
        wp_host = wp_host.astype(ml_dtypes.bfloat16)
        xT = xT.astype(ml_dtypes.bfloat16)
    return xT, wp_host, bias_host


def _run(user_batch, filt, W_k, b_k, trace=False):
    xT, wp_host, bias_host = _prep_inputs(user_batch, filt, W_k, b_k)
    nc = _build_program()
    in_maps = [
        {"xT": xT[c * BP:(c + 1) * BP], "wp": wp_host, "bias": bias_host}
        for c in range(N_CORES)
    ]
    res = run_bass_kernel_spmd(nc, in_maps, list(range(N_CORES)), trace=trace)
    outT = np.concatenate(
        [np.asarray(r["outT"], dtype=np.float32) for r in res.results], axis=0)
    out = outT.reshape(B, K, N_WIN).transpose(0, 2, 1)               # [B, N_WIN, K]
    return np.ascontiguousarray(out), res


def kernel(user_batch, filt, W_k, b_k):
    out, _ = _run(user_batch, filt, W_k, b_k, trace=False)
    return out
